# revision 56
# baseline (speedup 1.0000x reference)
"""Trainium2 Bass kernel for nn_Actor_metapop1_MDP.

Computes, for a batch of B=8192 states:
  logits = MLP(state)                      # 1025 -> 1024 -> 1024 -> 1026
  (logp, ent) = sampling-without-replacement log-prob/entropy over two
  heads (R = logits[:, :513], S = logits[:, 513:1026]) with K=32 steps.
Returns (2, B): row 0 = logpR+logpS, row 1 = entR+entS.

Strategy: pure data parallelism over 8 NeuronCores (1024 rows each).
Each core:
  - TensorE: transpose state (bf16), 3 matmul layers with bias folded
    into an appended ones-row / bias-row contraction chunk.
  - The per-step softmax scan is reformulated: with Z_t = sum of exp over
    the remaining set and S_t = sum of exp*logit, each step removes one
    gathered logit, so Z/S follow prefix sums of the gathered exps.
    logp = sum M_t*(g_t - log Z_t); ent = sum M_t*(log Z_t - S_t/Z_t).
  - GPSIMD ap_gather produces the per-row gathered logits: the 16-wrapped
    index layout means index list entry (s*16+q) for partition group
    [16g, 16g+16) reads idxs[16g+q, s]; keeping only q == p%16 via a
    constant diagonal mask yields a true per-partition gather.
  - VectorE tensor_tensor_scan gives the segmented exclusive prefix sums.
All index/mask/constant tensors are host-precomputed from the int inputs.
"""

import numpy as np

B, DIN, H, NACT = 8192, 1025, 1024, 1026
NH, K = 513, 32
NCORES = 8
BC = B // NCORES          # rows per core = 1024
R = BC // 128             # row tiles per core = 8
DIN_E = DIN + 1           # 1026 = state + ones column
NSEG = R * 2              # (row-tile, head) segments = 16
SFREE = NSEG * K          # 512
NELEMS = R * NACT         # 8208 flat logits per partition row-group

_CACHE = {}


def _build_nc():
    import concourse.bass as bass
    import concourse.mybir as mybir
    import concourse.tile as tile
    from concourse import bacc
    from concourse.masks import make_identity

    f32 = mybir.dt.float32
    bf16 = mybir.dt.bfloat16
    f8 = mybir.dt.float8e4
    u32 = mybir.dt.uint32
    i16 = mybir.dt.int16
    DR = mybir.MatmulPerfMode.DoubleRow
    AF = mybir.ActivationFunctionType
    OP = mybir.AluOpType
    AX = mybir.AxisListType

    # Force the activation-table pass to use the one set that contains BOTH
    # Exp and Ln (plus relu/copy/identity) — otherwise it alternates between
    # an exp-only and an ln-only table, costing ~1.3us per switch, 16x.
    from concourse import hw_specs

    def _combined_tables(arch, _orig=hw_specs.get_activation_tables):
        t = _orig(arch)
        for name, s in t.items():
            if name != "natural_log_exp_and_others":
                s.discard(mybir.ActivationFunctionType.Exp)
                s.discard(mybir.ActivationFunctionType.Ln)
        return t

    bacc.get_activation_tables = _combined_tables

    nc = bacc.Bacc()

    statee = nc.declare_dram_parameter("statee", [BC, DIN_E], bf16, isOutput=False)
    w0e8 = nc.declare_dram_parameter("w0e8", [128, 8 * H], f8, isOutput=False)
    w0et = nc.declare_dram_parameter("w0et", [2, H], bf16, isOutput=False)
    w1e8 = nc.declare_dram_parameter("w1e8", [128, 8 * H], f8, isOutput=False)
    w1et = nc.declare_dram_parameter("w1et", [1, H], bf16, isOutput=False)
    w2e8 = nc.declare_dram_parameter("w2e8", [128, 8 * NACT], f8, isOutput=False)
    w2et = nc.declare_dram_parameter("w2et", [1, NACT], bf16, isOutput=False)
    idxs_ext = nc.declare_dram_parameter("idxs", [128, SFREE], i16, isOutput=False)
    mmask_ext = nc.declare_dram_parameter("mmask", [128, SFREE], f32, isOutput=False)
    reset_ext = nc.declare_dram_parameter("reset", [128, SFREE], f32, isOutput=False)
    diag_ext = nc.declare_dram_parameter("diag", [128, 16], f32, isOutput=False)
    out_ext = nc.declare_dram_parameter("out", [2, BC], f32, isOutput=True)

    with tile.TileContext(nc) as tc:
        with (
            tc.tile_pool(name="consts", bufs=1) as cpool,
            tc.tile_pool(name="weights", bufs=1) as wpool,
            tc.tile_pool(name="acts", bufs=1) as apool,
            tc.tile_pool(name="samp", bufs=1) as mpool,
            tc.tile_pool(name="escr", bufs=2) as epool,
            tc.tile_pool(name="ptr", bufs=2, space="PSUM") as ptr_pool,
            tc.tile_pool(name="pmm", bufs=6, space="PSUM") as pmm_pool,
        ):
            # ---------- constants ----------
            identity = cpool.tile([128, 128], bf16, tag="identity")
            make_identity(nc, identity[:, :])
            ones_row = cpool.tile([1, H], bf16, tag="ones_row")
            nc.vector.memset(ones_row[:, :], 1.0)

            # state first (the transposes and layer 1 gate on it); one DMA
            # per row-tile so transposes pipeline behind the first chunk
            s_in = mpool.tile([128, R * DIN_E], bf16, tag="graw", name="s_in")
            for r in range(R):
                nc.sync.dma_start(
                    out=s_in[:, r * DIN_E : (r + 1) * DIN_E],
                    in_=statee[r * 128 : (r + 1) * 128, :],
                )

            # ---------- weights ----------
            # w0/w1 as fp8 (x16 scaled on host), [p, kchunk, m] layout for
            # DoubleRow matmuls; bias/tail rows in bf16
            w0f8 = wpool.tile([128, 8 * H], f8, tag="w0f8")
            nc.scalar.dma_start(out=w0f8[:, :], in_=w0e8[:, :])
            w0tail = wpool.tile([2, H], bf16, tag="w0tail")
            nc.scalar.dma_start(out=w0tail[:, :], in_=w0et[:, :])
            w1f8 = wpool.tile([128, 8 * H], f8, tag="w1f8")
            nc.scalar.dma_start(out=w1f8[:, :], in_=w1e8[:, :])
            w1tail = wpool.tile([1, H], bf16, tag="w1tail")
            nc.scalar.dma_start(out=w1tail[:, :], in_=w1et[:, :])
            w2f8 = wpool.tile([128, 8 * NACT], f8, tag="w2f8")
            nc.scalar.dma_start(out=w2f8[:, :], in_=w2e8[:, :])
            w2tail = wpool.tile([1, NACT], bf16, tag="w2tail")
            nc.scalar.dma_start(out=w2tail[:, :], in_=w2et[:, :])

            idxs_sb = cpool.tile([128, SFREE], i16, tag="idxs")
            nc.sync.dma_start(out=idxs_sb[:, :], in_=idxs_ext[:, :])
            mmask_sb = cpool.tile([128, SFREE], f32, tag="mmask")
            nc.sync.dma_start(out=mmask_sb[:, :], in_=mmask_ext[:, :])
            reset_sb = cpool.tile([128, SFREE], f32, tag="reset")
            nc.sync.dma_start(out=reset_sb[:, :], in_=reset_ext[:, :])
            diag_sb = cpool.tile([128, 16], f32, tag="diag")
            nc.sync.dma_start(out=diag_sb[:, :], in_=diag_ext[:, :])

            # ---------- state load + transpose ----------
            # stateT: fp8 packed [p, d, b] for DoubleRow rhs; tail (2, BC)
            # bf16 = [state col 1024; ones]
            stT8f = apool.tile([128, 8 * BC], f8, tag="stT8f")
            stT8v = stT8f[:, :].rearrange("p (d b) -> p d b", b=BC)
            stTtail = apool.tile([2, BC], bf16, tag="stTtail")

            for r in range(R):
                for d in range(8):
                    pt = ptr_pool.tile([128, 128], bf16, tag="ptr")
                    nc.tensor.transpose(
                        pt[:, :],
                        s_in[:, r * DIN_E + d * 128 : r * DIN_E + (d + 1) * 128],
                        identity[:, :],
                    )
                    nc.vector.tensor_copy(
                        stT8v[:, d, r * 128 : (r + 1) * 128], pt[:, :]
                    )
                pt = ptr_pool.tile([2, 128], bf16, tag="ptr")
                nc.tensor.transpose(
                    pt[:, :],
                    s_in[:, r * DIN_E + 1024 : r * DIN_E + 1026],
                    identity[:, :],
                )
                nc.vector.tensor_copy(stTtail[:, r * 128 : (r + 1) * 128], pt[:, :])

            # ---------- MLP pipelined by batch halves ----------
            # Sampling (DVE-heavy) for half 0 overlaps layers 1-3 of half 1,
            # so the vector engine never backlogs past the end of PE work.
            h08 = apool.tile([128, 8 * BC], f8, tag="h08")
            h08v = h08[:, :].rearrange("p (d b) -> p d b", b=BC)
            w0f8v = w0f8[:, :].rearrange("p (d m) -> p d m", m=H)
            w1f8v = w1f8[:, :].rearrange("p (d m) -> p d m", m=H)
            w2f8v = w2f8[:, :].rearrange("p (d n) -> p d n", n=NACT)
            h18 = apool.tile([128, 8 * BC], f8, tag="h18")
            h18v = h18[:, :].rearrange("p (d b) -> p d b", b=BC)

            z0 = mpool.tile([128, NSEG], f32, tag="z0")
            s0 = mpool.tile([128, NSEG], f32, tag="s0")
            gsel = mpool.tile([128, SFREE], f32, tag="gsel")
            iscr = mpool.tile([128, 1], i16, tag="iscr")
            CS = SFREE // R  # 64 s-values per row-tile chunk
            CW = 342         # 3 * 342 = 1026

            ew = mpool.tile([128, SFREE], f32, tag="ew")
            gm = mpool.tile([128, SFREE], f32, tag="gm")
            wg = mpool.tile([128, SFREE], f32, tag="wg")
            cumw = mpool.tile([128, SFREE], f32, tag="cumw")
            cumwg = mpool.tile([128, SFREE], f32, tag="cumwg")
            zt = mpool.tile([128, SFREE], f32, tag="zt")
            stt = mpool.tile([128, SFREE], f32, tag="stt")
            logz = mpool.tile([128, SFREE], f32, tag="logz")
            rz = mpool.tile([128, SFREE], f32, tag="rz")
            srz = mpool.tile([128, SFREE], f32, tag="srz")
            mlz = mpool.tile([128, SFREE], f32, tag="mlz")
            lpred = mpool.tile([128, NSEG], f32, tag="lpred")
            entred2 = mpool.tile([128, NSEG], f32, tag="entred2")

            def recurrence_chunk(c, span=1):
                # `span` row-tiles of 64 columns (2 segments of 32 steps each)
                # inclusive scans; exclusive prefix recovered as cum - w:
                #   Z_t = Z0 - (cum_t - w_t) = (Z0 - cum_t) + w_t
                lo, hi = c * CS, (c + span) * CS
                flo, fhi = c * 2, (c + span) * 2
                FC = 2 * span
                g_c = gsel[:, lo:hi]
                m_c = mmask_sb[:, lo:hi]
                nc.scalar.activation(ew[:, lo:hi], g_c, AF.Exp)
                nc.vector.tensor_tensor(ew[:, lo:hi], ew[:, lo:hi], m_c, OP.mult)
                nc.vector.tensor_tensor(wg[:, lo:hi], ew[:, lo:hi], g_c, OP.mult)
                nc.vector.tensor_tensor_scan(
                    cumw[:, lo:hi], reset_sb[:, lo:hi], ew[:, lo:hi],
                    0.0, OP.mult, OP.add,
                )
                nc.vector.tensor_tensor_scan(
                    cumwg[:, lo:hi], reset_sb[:, lo:hi], wg[:, lo:hi],
                    0.0, OP.mult, OP.add,
                )
                z0b = z0[:, flo:fhi].rearrange("p f -> p f ()").broadcast_to(
                    (128, FC, K)
                )
                s0b = s0[:, flo:fhi].rearrange("p f -> p f ()").broadcast_to(
                    (128, FC, K)
                )
                zt3 = zt[:, lo:hi].rearrange("p (f t) -> p f t", t=K)
                st3 = stt[:, lo:hi].rearrange("p (f t) -> p f t", t=K)
                nc.vector.tensor_tensor(
                    zt3, z0b, cumw[:, lo:hi].rearrange("p (f t) -> p f t", t=K),
                    OP.subtract,
                )
                nc.vector.tensor_tensor(
                    zt[:, lo:hi], zt[:, lo:hi], ew[:, lo:hi], OP.add
                )
                nc.vector.tensor_tensor(
                    st3, s0b, cumwg[:, lo:hi].rearrange("p (f t) -> p f t", t=K),
                    OP.subtract,
                )
                nc.vector.tensor_tensor(
                    stt[:, lo:hi], stt[:, lo:hi], wg[:, lo:hi], OP.add
                )
                nc.scalar.activation(logz[:, lo:hi], zt[:, lo:hi], AF.Ln)
                nc.vector.reciprocal(rz[:, lo:hi], zt[:, lo:hi])
                # srz = St/Zt ; d1 = g - logZ ; d2 = logZ - srz; mask; reduce
                nc.vector.tensor_tensor(
                    srz[:, lo:hi], stt[:, lo:hi], rz[:, lo:hi], OP.mult
                )
                nc.vector.tensor_tensor(
                    gm[:, lo:hi], g_c, logz[:, lo:hi], OP.subtract
                )
                nc.vector.tensor_tensor(
                    mlz[:, lo:hi], logz[:, lo:hi], srz[:, lo:hi], OP.subtract
                )
                nc.vector.tensor_tensor(gm[:, lo:hi], gm[:, lo:hi], m_c, OP.mult)
                nc.vector.tensor_tensor(mlz[:, lo:hi], mlz[:, lo:hi], m_c, OP.mult)
                nc.vector.tensor_reduce(
                    lpred[:, flo:fhi],
                    gm[:, lo:hi].rearrange("p (f t) -> p f t", t=K),
                    AX.X,
                    OP.add,
                )
                nc.vector.tensor_reduce(
                    entred2[:, flo:fhi],
                    mlz[:, lo:hi].rearrange("p (f t) -> p f t", t=K),
                    AX.X,
                    OP.add,
                )
                # head-sum + output columns for these row tiles, so the
                # final output DMA isn't serialized at the very end
                lp1 = mpool.tile([128, span], f32, tag=f"lp1_{c}", name=f"lp1_{c}")
                en1 = mpool.tile([128, span], f32, tag=f"en1_{c}", name=f"en1_{c}")
                nc.vector.tensor_reduce(
                    lp1[:, :],
                    lpred[:, flo:fhi].rearrange("p (r h) -> p r h", h=2),
                    AX.X,
                    OP.add,
                )
                nc.vector.tensor_reduce(
                    en1[:, :],
                    entred2[:, flo:fhi].rearrange("p (r h) -> p r h", h=2),
                    AX.X,
                    OP.add,
                )
                nc.sync.dma_start(
                    out=out_ext[0, c * 128 : (c + span) * 128].rearrange(
                        "(x p) -> p x", p=128
                    ),
                    in_=lp1[:, :],
                )
                nc.sync.dma_start(
                    out=out_ext[1, c * 128 : (c + span) * 128].rearrange(
                        "(x p) -> p x", p=128
                    ),
                    in_=en1[:, :],
                )

            # Asymmetric portions: the last portions are small so their
            # (DVE-bound) sampling never backlogs far past the end of PE work.
            PORTIONS = [(0, 4), (4, 7), (7, 8)]
            for b0, b1 in PORTIONS:
                hlo = b0 * 128
                W = (b1 - b0) * 128
                # ----- layer 1 (this portion): 4 fp8 DoubleRow + bf16 tail -----
                for hc in range(8):
                    ps = pmm_pool.tile([128, W], f32, tag="pmm")
                    for dd in range(4):
                        nc.tensor.matmul(
                            ps[:, :],
                            w0f8v[:, 2 * dd : 2 * dd + 2, hc * 128 : (hc + 1) * 128],
                            stT8v[:, 2 * dd : 2 * dd + 2, hlo : hlo + W],
                            start=(dd == 0),
                            stop=False,
                            perf_mode=DR,
                        )
                    nc.tensor.matmul(
                        ps[:, :],
                        w0tail[:, hc * 128 : (hc + 1) * 128],
                        stTtail[:, hlo : hlo + W],
                        start=False,
                        stop=True,
                    )
                    nc.scalar.activation(
                        h08v[:, hc, hlo : hlo + W], ps[:, :], AF.Relu,
                        scale=1.0 / 16.0,
                    )
                # ----- layer 2 (this portion): 4 fp8 DoubleRow + bias tail -----
                for hc in range(8):
                    ps = pmm_pool.tile([128, W], f32, tag="pmm")
                    for dd in range(4):
                        nc.tensor.matmul(
                            ps[:, :],
                            w1f8v[:, 2 * dd : 2 * dd + 2, hc * 128 : (hc + 1) * 128],
                            h08v[:, 2 * dd : 2 * dd + 2, hlo : hlo + W],
                            start=(dd == 0),
                            stop=False,
                            perf_mode=DR,
                        )
                    nc.tensor.matmul(
                        ps[:, :],
                        w1tail[:, hc * 128 : (hc + 1) * 128],
                        ones_row[:, hlo : hlo + W],
                        start=False,
                        stop=True,
                    )
                    nc.scalar.activation(
                        h18v[:, hc, hlo : hlo + W], ps[:, :], AF.Relu,
                        scale=1.0 / 16.0,
                    )
                # ----- layer 3 + sampling (this portion), per row tile -----
                for bt in range(b0, b1):
                    logits_bt = mpool.tile(
                        [128, NACT], f32, tag="logits", bufs=3, name=f"logits{bt}"
                    )
                    for cc in range(3):
                        ps = pmm_pool.tile([128, CW], f32, tag="pmm")
                        for dd in range(4):
                            nc.tensor.matmul(
                                ps[:, :],
                                h18v[:, 2 * dd : 2 * dd + 2, bt * 128 : (bt + 1) * 128],
                                w2f8v[:, 2 * dd : 2 * dd + 2, cc * CW : (cc + 1) * CW],
                                start=(dd == 0),
                                stop=False,
                                perf_mode=DR,
                            )
                        nc.tensor.matmul(
                            ps[:, :],
                            ones_row[:, bt * 128 : (bt + 1) * 128],
                            w2tail[:, cc * CW : (cc + 1) * CW],
                            start=False,
                            stop=True,
                        )
                        nc.scalar.mul(
                            logits_bt[:, cc * CW : (cc + 1) * CW], ps[:, :],
                            1.0 / 16.0,
                        )
                    lsl = logits_bt[:, :]
                    # head sums: Z0 = sum exp, S0 = sum exp*logit
                    etile = epool.tile([128, NACT], f32, tag="etile")
                    for h in range(2):
                        f = bt * 2 + h
                        nc.scalar.activation(
                            etile[:, h * NH : (h + 1) * NH],
                            lsl[:, h * NH : (h + 1) * NH],
                            AF.Exp,
                            accum_out=z0[:, f : f + 1],
                        )
                        nc.vector.tensor_tensor(
                            etile[:, h * NH : (h + 1) * NH],
                            etile[:, h * NH : (h + 1) * NH],
                            lsl[:, h * NH : (h + 1) * NH],
                            OP.mult,
                        )
                        nc.vector.tensor_reduce(
                            s0[:, f : f + 1],
                            etile[:, h * NH : (h + 1) * NH],
                            AX.X,
                            OP.add,
                        )
                    # gather this row tile's chosen logits
                    graw = mpool.tile([128, CS * 16], f32, tag="grawc", bufs=2)
                    ltouch = lsl.rearrange("p (x c) -> p x c", c=CW)[:, :, 0:1]
                    nc.gpsimd.tensor_copy(
                        graw[:, 0:3].rearrange("p (x c) -> p x c", c=1), ltouch
                    )
                    nc.gpsimd.tensor_copy(
                        iscr[:, :], idxs_sb[:, bt * CS : bt * CS + 1]
                    )
                    nc.gpsimd.ap_gather(
                        out_ap=graw[:, :],
                        in_ap=lsl,
                        idxs_ap=idxs_sb[:, bt * CS : (bt + 1) * CS],
                        channels=128,
                        num_elems=NACT,
                        d=1,
                        num_idxs=CS * 16,
                    )
                    graw3 = graw[:, :].rearrange("p (s q) -> p s q", q=16)
                    diag3 = diag_sb[:, :].rearrange("p q -> p () q").broadcast_to(
                        (128, CS, 16)
                    )
                    nc.gpsimd.tensor_tensor(graw3, graw3, diag3, OP.mult)
                    nc.vector.tensor_reduce(
                        gsel[:, bt * CS : (bt + 1) * CS], graw3, AX.X, OP.add
                    )
                    if b1 - b0 == 4:
                        # wide portion: pair row-tiles to halve DVE op count
                        if bt % 2 == 1:
                            recurrence_chunk(bt - 1, span=2)
                    else:
                        recurrence_chunk(bt)


    nc.compile()
    return nc


def _get_nc():
    if "nc" not in _CACHE:
        _CACHE["nc"] = _build_nc()
    return _CACHE["nc"]


def _prep_shared(W0, b0, W1, b1, W2, b2):
    import ml_dtypes

    bf = ml_dtypes.bfloat16
    f8 = ml_dtypes.float8_e4m3
    FS = 16.0  # fp8 weight pre-scale; undone by the relu-copy's 1/16
    w0e8 = (
        (W0[:1024] * FS).reshape(8, 128, H).transpose(1, 0, 2).reshape(128, 8 * H)
    ).astype(f8)
    w0et = np.vstack([W0[1024:1025] * FS, b0[None, :] * FS]).astype(bf)  # (2, H)
    w1e8 = (
        (W1 * FS).reshape(8, 128, H).transpose(1, 0, 2).reshape(128, 8 * H)
    ).astype(f8)
    w1et = (b1[None, :] * FS).astype(bf)                     # (1, H)
    w2e8 = (
        (W2 * FS).reshape(8, 128, NACT).transpose(1, 0, 2).reshape(128, 8 * NACT)
    ).astype(f8)
    w2et = (b2[None, :] * FS).astype(bf)                     # (1, NACT)
    t = np.arange(K, dtype=np.int32)
    reset = np.broadcast_to(
        (t != 0).astype(np.float32), (128, NSEG, K)
    ).reshape(128, SFREE).copy()
    q = np.arange(16)
    diag = (q[None, :] == (np.arange(128) % 16)[:, None]).astype(np.float32)
    return (w0e8, w0et, w1e8, w1et, w2e8, w2et), reset, diag


def _prep_core(c, state, seq_idx_R, seq_len_R, seq_idx_S, seq_len_S):
    import ml_dtypes

    bf = ml_dtypes.bfloat16
    rows = slice(c * BC, (c + 1) * BC)
    st = state[rows].astype(np.float32)
    statee = np.concatenate(
        [st, np.ones((BC, 1), np.float32)], axis=1
    ).astype(bf)                                             # (1024, 1026)

    idxR = np.maximum(seq_idx_R[rows].astype(np.int32), 0)
    idxS = np.maximum(seq_idx_S[rows].astype(np.int32), 0) + NH
    idx_all = np.stack([idxR, idxS], axis=1)                 # (1024, 2, 32)
    enc = idx_all.reshape(R, 128, 2, K)
    idxs = enc.transpose(1, 0, 2, 3).reshape(128, SFREE).astype(np.int16)

    lens = np.stack([seq_len_R[rows], seq_len_S[rows]], axis=1)  # (1024, 2)
    t = np.arange(K, dtype=np.int32)
    act = (t[None, None, :] < lens[:, :, None]).astype(np.float32)
    mmask = (
        act.reshape(R, 128, 2, K).transpose(1, 0, 2, 3).reshape(128, SFREE)
    ).astype(np.float32)
    return statee, idxs, mmask


def kernel(
    state,
    W0,
    b0,
    W1,
    b1,
    W2,
    b2,
    seq_idx_R,
    seq_len_R,
    seq_idx_S,
    seq_len_S,
    _return_extras=False,
):
    from concourse.bass_utils import run_bass_kernel_spmd

    nc = _get_nc()
    (w0e8, w0et, w1e8, w1et, w2e8, w2et), reset, diag = _prep_shared(
        np.asarray(W0), np.asarray(b0), np.asarray(W1),
        np.asarray(b1), np.asarray(W2), np.asarray(b2),
    )
    in_maps = []
    for c in range(NCORES):
        statee, idxs, mmask = _prep_core(
            c,
            np.asarray(state),
            np.asarray(seq_idx_R),
            np.asarray(seq_len_R),
            np.asarray(seq_idx_S),
            np.asarray(seq_len_S),
        )
        in_maps.append(
            {
                "statee": statee,
                "w0e8": w0e8,
                "w0et": w0et,
                "w1e8": w1e8,
                "w1et": w1et,
                "w2e8": w2e8,
                "w2et": w2et,
                "idxs": idxs,
                "mmask": mmask,
                "reset": reset,
                "diag": diag,
            }
        )
    res = run_bass_kernel_spmd(nc, in_maps, core_ids=list(range(NCORES)))
    out = np.concatenate(
        [res.results[c]["out"].astype(np.float32) for c in range(NCORES)], axis=1
    )
    if _return_extras:
        return out, res
    return out


# revision 59
# speedup vs baseline: 1.0345x; 1.0345x over previous
"""Trainium2 Bass kernel for nn_Actor_metapop1_MDP.

Computes, for a batch of B=8192 states:
  logits = MLP(state)                      # 1025 -> 1024 -> 1024 -> 1026
  (logp, ent) = sampling-without-replacement log-prob/entropy over two
  heads (R = logits[:, :513], S = logits[:, 513:1026]) with K=32 steps.
Returns (2, B): row 0 = logpR+logpS, row 1 = entR+entS.

Strategy: pure data parallelism over 8 NeuronCores (1024 rows each).
Each core:
  - TensorE: transpose state (bf16), 3 matmul layers with bias folded
    into an appended ones-row / bias-row contraction chunk.
  - The per-step softmax scan is reformulated: with Z_t = sum of exp over
    the remaining set and S_t = sum of exp*logit, each step removes one
    gathered logit, so Z/S follow prefix sums of the gathered exps.
    logp = sum M_t*(g_t - log Z_t); ent = sum M_t*(log Z_t - S_t/Z_t).
  - GPSIMD ap_gather produces the per-row gathered logits: the 16-wrapped
    index layout means index list entry (s*16+q) for partition group
    [16g, 16g+16) reads idxs[16g+q, s]; keeping only q == p%16 via a
    constant diagonal mask yields a true per-partition gather.
  - VectorE tensor_tensor_scan gives the segmented exclusive prefix sums.
All index/mask/constant tensors are host-precomputed from the int inputs.
"""

import numpy as np

B, DIN, H, NACT = 8192, 1025, 1024, 1026
NH, K = 513, 32
NCORES = 8
BC = B // NCORES          # rows per core = 1024
R = BC // 128             # row tiles per core = 8
DIN_E = DIN + 1           # 1026 = state + ones column
NSEG = R * 2              # (row-tile, head) segments = 16
SFREE = NSEG * K          # 512
NELEMS = R * NACT         # 8208 flat logits per partition row-group

_CACHE = {}


def _build_nc():
    import concourse.bass as bass
    import concourse.mybir as mybir
    import concourse.tile as tile
    from concourse import bacc
    from concourse.masks import make_identity

    f32 = mybir.dt.float32
    bf16 = mybir.dt.bfloat16
    f8 = mybir.dt.float8e4
    u32 = mybir.dt.uint32
    i16 = mybir.dt.int16
    DR = mybir.MatmulPerfMode.DoubleRow
    AF = mybir.ActivationFunctionType
    OP = mybir.AluOpType
    AX = mybir.AxisListType

    # Force the activation-table pass to use the one set that contains BOTH
    # Exp and Ln (plus relu/copy/identity) — otherwise it alternates between
    # an exp-only and an ln-only table, costing ~1.3us per switch, 16x.
    from concourse import hw_specs

    def _combined_tables(arch, _orig=hw_specs.get_activation_tables):
        t = _orig(arch)
        for name, s in t.items():
            if name != "natural_log_exp_and_others":
                s.discard(mybir.ActivationFunctionType.Exp)
                s.discard(mybir.ActivationFunctionType.Ln)
        return t

    bacc.get_activation_tables = _combined_tables

    nc = bacc.Bacc()

    statee = nc.declare_dram_parameter("statee", [BC, DIN_E], bf16, isOutput=False)
    w0e8 = nc.declare_dram_parameter("w0e8", [128, 8 * H], f8, isOutput=False)
    w0et = nc.declare_dram_parameter("w0et", [2, H], bf16, isOutput=False)
    w1e8 = nc.declare_dram_parameter("w1e8", [128, 8 * H], f8, isOutput=False)
    w1et = nc.declare_dram_parameter("w1et", [1, H], bf16, isOutput=False)
    w2e8 = nc.declare_dram_parameter("w2e8", [128, 8 * NACT], f8, isOutput=False)
    w2et = nc.declare_dram_parameter("w2et", [1, NACT], bf16, isOutput=False)
    idxs_ext = nc.declare_dram_parameter("idxs", [128, SFREE], i16, isOutput=False)
    mmask_ext = nc.declare_dram_parameter("mmask", [128, SFREE], f32, isOutput=False)
    reset_ext = nc.declare_dram_parameter("reset", [128, SFREE], f32, isOutput=False)
    diag_ext = nc.declare_dram_parameter("diag", [128, 16], f32, isOutput=False)
    out_ext = nc.declare_dram_parameter("out", [2, BC], f32, isOutput=True)

    with tile.TileContext(nc) as tc:
        with (
            tc.tile_pool(name="consts", bufs=1) as cpool,
            tc.tile_pool(name="weights", bufs=1) as wpool,
            tc.tile_pool(name="acts", bufs=1) as apool,
            tc.tile_pool(name="samp", bufs=1) as mpool,
            tc.tile_pool(name="escr", bufs=2) as epool,
            tc.tile_pool(name="ptr", bufs=3, space="PSUM") as ptr_pool,
            tc.tile_pool(name="pmm", bufs=5, space="PSUM") as pmm_pool,
        ):
            # ---------- constants ----------
            identity = cpool.tile([128, 128], bf16, tag="identity")
            make_identity(nc, identity[:, :])
            ones_row = cpool.tile([1, H], bf16, tag="ones_row")
            nc.vector.memset(ones_row[:, :], 1.0)

            # state first (the transposes and layer 1 gate on it); one DMA
            # per row-tile so transposes pipeline behind the first chunk
            s_in = mpool.tile([128, R * DIN_E], bf16, tag="graw", name="s_in")
            for r in range(R):
                nc.sync.dma_start(
                    out=s_in[:, r * DIN_E : (r + 1) * DIN_E],
                    in_=statee[r * 128 : (r + 1) * 128, :],
                )

            # ---------- weights ----------
            # w0/w1 as fp8 (x16 scaled on host), [p, kchunk, m] layout for
            # DoubleRow matmuls; bias/tail rows in bf16
            w0f8 = wpool.tile([128, 8 * H], f8, tag="w0f8")
            nc.scalar.dma_start(out=w0f8[:, :], in_=w0e8[:, :])
            w0tail = wpool.tile([2, H], bf16, tag="w0tail")
            nc.scalar.dma_start(out=w0tail[:, :], in_=w0et[:, :])
            w1f8 = wpool.tile([128, 8 * H], f8, tag="w1f8")
            nc.scalar.dma_start(out=w1f8[:, :], in_=w1e8[:, :])
            w1tail = wpool.tile([1, H], bf16, tag="w1tail")
            nc.scalar.dma_start(out=w1tail[:, :], in_=w1et[:, :])
            w2f8 = wpool.tile([128, 8 * NACT], f8, tag="w2f8")
            nc.scalar.dma_start(out=w2f8[:, :], in_=w2e8[:, :])
            w2tail = wpool.tile([1, NACT], bf16, tag="w2tail")
            nc.scalar.dma_start(out=w2tail[:, :], in_=w2et[:, :])

            idxs_sb = cpool.tile([128, SFREE], i16, tag="idxs")
            nc.sync.dma_start(out=idxs_sb[:, :], in_=idxs_ext[:, :])
            mmask_sb = cpool.tile([128, SFREE], f32, tag="mmask")
            nc.sync.dma_start(out=mmask_sb[:, :], in_=mmask_ext[:, :])
            reset_sb = cpool.tile([128, SFREE], f32, tag="reset")
            nc.sync.dma_start(out=reset_sb[:, :], in_=reset_ext[:, :])
            diag_sb = cpool.tile([128, 16], f32, tag="diag")
            nc.sync.dma_start(out=diag_sb[:, :], in_=diag_ext[:, :])

            # ---------- state load + transpose ----------
            # stateT: fp8 packed [p, d, b] for DoubleRow rhs; tail (2, BC)
            # bf16 = [state col 1024; ones]
            stT8f = apool.tile([128, 8 * BC], f8, tag="stT8f")
            stT8v = stT8f[:, :].rearrange("p (d b) -> p d b", b=BC)
            stTtail = apool.tile([2, BC], bf16, tag="stTtail")

            for r in range(R):
                for d in range(8):
                    pt = ptr_pool.tile([128, 128], bf16, tag="ptr")
                    nc.tensor.transpose(
                        pt[:, :],
                        s_in[:, r * DIN_E + d * 128 : r * DIN_E + (d + 1) * 128],
                        identity[:, :],
                    )
                    nc.vector.tensor_copy(
                        stT8v[:, d, r * 128 : (r + 1) * 128], pt[:, :]
                    )
                pt = ptr_pool.tile([2, 128], bf16, tag="ptr")
                nc.tensor.transpose(
                    pt[:, :],
                    s_in[:, r * DIN_E + 1024 : r * DIN_E + 1026],
                    identity[:, :],
                )
                nc.vector.tensor_copy(stTtail[:, r * 128 : (r + 1) * 128], pt[:, :])

            # ---------- MLP pipelined by batch halves ----------
            # Sampling (DVE-heavy) for half 0 overlaps layers 1-3 of half 1,
            # so the vector engine never backlogs past the end of PE work.
            h08 = apool.tile([128, 8 * BC], f8, tag="h08")
            h08v = h08[:, :].rearrange("p (d b) -> p d b", b=BC)
            w0f8v = w0f8[:, :].rearrange("p (d m) -> p d m", m=H)
            w1f8v = w1f8[:, :].rearrange("p (d m) -> p d m", m=H)
            w2f8v = w2f8[:, :].rearrange("p (d n) -> p d n", n=NACT)
            h18 = apool.tile([128, 8 * BC], f8, tag="h18")
            h18v = h18[:, :].rearrange("p (d b) -> p d b", b=BC)

            z0 = mpool.tile([128, NSEG], f32, tag="z0")
            s0 = mpool.tile([128, NSEG], f32, tag="s0")
            gsel = mpool.tile([128, SFREE], f32, tag="gsel")
            iscr = mpool.tile([128, 1], i16, tag="iscr")
            CS = SFREE // R  # 64 s-values per row-tile chunk
            CW = 342         # 3 * 342 = 1026

            ew = mpool.tile([128, SFREE], f32, tag="ew")
            gm = mpool.tile([128, SFREE], f32, tag="gm")
            wg = mpool.tile([128, SFREE], f32, tag="wg")
            cumw = mpool.tile([128, SFREE], f32, tag="cumw")
            cumwg = mpool.tile([128, SFREE], f32, tag="cumwg")
            zt = mpool.tile([128, SFREE], f32, tag="zt")
            stt = mpool.tile([128, SFREE], f32, tag="stt")
            logz = mpool.tile([128, SFREE], f32, tag="logz")
            rz = mpool.tile([128, SFREE], f32, tag="rz")
            srz = mpool.tile([128, SFREE], f32, tag="srz")
            mlz = mpool.tile([128, SFREE], f32, tag="mlz")
            lpred = mpool.tile([128, NSEG], f32, tag="lpred")
            entred2 = mpool.tile([128, NSEG], f32, tag="entred2")

            def recurrence_chunk(c, span=1):
                # `span` row-tiles of 64 columns (2 segments of 32 steps each)
                # inclusive scans; exclusive prefix recovered as cum - w:
                #   Z_t = Z0 - (cum_t - w_t) = (Z0 - cum_t) + w_t
                lo, hi = c * CS, (c + span) * CS
                flo, fhi = c * 2, (c + span) * 2
                FC = 2 * span
                g_c = gsel[:, lo:hi]
                m_c = mmask_sb[:, lo:hi]
                nc.scalar.activation(ew[:, lo:hi], g_c, AF.Exp)
                nc.vector.tensor_tensor(ew[:, lo:hi], ew[:, lo:hi], m_c, OP.mult)
                nc.vector.tensor_tensor(wg[:, lo:hi], ew[:, lo:hi], g_c, OP.mult)
                nc.vector.tensor_tensor_scan(
                    cumw[:, lo:hi], reset_sb[:, lo:hi], ew[:, lo:hi],
                    0.0, OP.mult, OP.add,
                )
                nc.vector.tensor_tensor_scan(
                    cumwg[:, lo:hi], reset_sb[:, lo:hi], wg[:, lo:hi],
                    0.0, OP.mult, OP.add,
                )
                z0b = z0[:, flo:fhi].rearrange("p f -> p f ()").broadcast_to(
                    (128, FC, K)
                )
                s0b = s0[:, flo:fhi].rearrange("p f -> p f ()").broadcast_to(
                    (128, FC, K)
                )
                zt3 = zt[:, lo:hi].rearrange("p (f t) -> p f t", t=K)
                st3 = stt[:, lo:hi].rearrange("p (f t) -> p f t", t=K)
                nc.vector.tensor_tensor(
                    zt3, z0b, cumw[:, lo:hi].rearrange("p (f t) -> p f t", t=K),
                    OP.subtract,
                )
                nc.vector.tensor_tensor(
                    zt[:, lo:hi], zt[:, lo:hi], ew[:, lo:hi], OP.add
                )
                nc.vector.tensor_tensor(
                    st3, s0b, cumwg[:, lo:hi].rearrange("p (f t) -> p f t", t=K),
                    OP.subtract,
                )
                nc.vector.tensor_tensor(
                    stt[:, lo:hi], stt[:, lo:hi], wg[:, lo:hi], OP.add
                )
                nc.scalar.activation(logz[:, lo:hi], zt[:, lo:hi], AF.Ln)
                nc.vector.reciprocal(rz[:, lo:hi], zt[:, lo:hi])
                # srz = St/Zt ; d1 = g - logZ ; d2 = logZ - srz; mask; reduce
                nc.vector.tensor_tensor(
                    srz[:, lo:hi], stt[:, lo:hi], rz[:, lo:hi], OP.mult
                )
                nc.vector.tensor_tensor(
                    gm[:, lo:hi], g_c, logz[:, lo:hi], OP.subtract
                )
                nc.vector.tensor_tensor(
                    mlz[:, lo:hi], logz[:, lo:hi], srz[:, lo:hi], OP.subtract
                )
                nc.vector.tensor_tensor(gm[:, lo:hi], gm[:, lo:hi], m_c, OP.mult)
                nc.vector.tensor_tensor(mlz[:, lo:hi], mlz[:, lo:hi], m_c, OP.mult)
                nc.vector.tensor_reduce(
                    lpred[:, flo:fhi],
                    gm[:, lo:hi].rearrange("p (f t) -> p f t", t=K),
                    AX.X,
                    OP.add,
                )
                nc.vector.tensor_reduce(
                    entred2[:, flo:fhi],
                    mlz[:, lo:hi].rearrange("p (f t) -> p f t", t=K),
                    AX.X,
                    OP.add,
                )
                # head-sum + output columns for these row tiles, so the
                # final output DMA isn't serialized at the very end
                lp1 = mpool.tile([128, span], f32, tag=f"lp1_{c}", name=f"lp1_{c}")
                en1 = mpool.tile([128, span], f32, tag=f"en1_{c}", name=f"en1_{c}")
                nc.vector.tensor_reduce(
                    lp1[:, :],
                    lpred[:, flo:fhi].rearrange("p (r h) -> p r h", h=2),
                    AX.X,
                    OP.add,
                )
                nc.vector.tensor_reduce(
                    en1[:, :],
                    entred2[:, flo:fhi].rearrange("p (r h) -> p r h", h=2),
                    AX.X,
                    OP.add,
                )
                nc.sync.dma_start(
                    out=out_ext[0, c * 128 : (c + span) * 128].rearrange(
                        "(x p) -> p x", p=128
                    ),
                    in_=lp1[:, :],
                )
                nc.sync.dma_start(
                    out=out_ext[1, c * 128 : (c + span) * 128].rearrange(
                        "(x p) -> p x", p=128
                    ),
                    in_=en1[:, :],
                )

            # Asymmetric portions: the last portions are small so their
            # (DVE-bound) sampling never backlogs far past the end of PE work.
            PORTIONS = [(0, 4), (4, 7), (7, 8)]
            for b0, b1 in PORTIONS:
                hlo = b0 * 128
                W = (b1 - b0) * 128
                # ----- layer 1 (this portion): 4 fp8 DoubleRow + bf16 tail -----
                for hc in range(8):
                    ps = pmm_pool.tile([128, W], f32, tag="pmm")
                    for dd in range(4):
                        nc.tensor.matmul(
                            ps[:, :],
                            w0f8v[:, 2 * dd : 2 * dd + 2, hc * 128 : (hc + 1) * 128],
                            stT8v[:, 2 * dd : 2 * dd + 2, hlo : hlo + W],
                            start=(dd == 0),
                            stop=False,
                            perf_mode=DR,
                        )
                    nc.tensor.matmul(
                        ps[:, :],
                        w0tail[:, hc * 128 : (hc + 1) * 128],
                        stTtail[:, hlo : hlo + W],
                        start=False,
                        stop=True,
                    )
                    nc.scalar.activation(
                        h08v[:, hc, hlo : hlo + W], ps[:, :], AF.Relu,
                        scale=1.0 / 16.0,
                    )
                # ----- layer 2 (this portion): 4 fp8 DoubleRow + bias tail -----
                for hc in range(8):
                    ps = pmm_pool.tile([128, W], f32, tag="pmm")
                    for dd in range(4):
                        nc.tensor.matmul(
                            ps[:, :],
                            w1f8v[:, 2 * dd : 2 * dd + 2, hc * 128 : (hc + 1) * 128],
                            h08v[:, 2 * dd : 2 * dd + 2, hlo : hlo + W],
                            start=(dd == 0),
                            stop=False,
                            perf_mode=DR,
                        )
                    nc.tensor.matmul(
                        ps[:, :],
                        w1tail[:, hc * 128 : (hc + 1) * 128],
                        ones_row[:, hlo : hlo + W],
                        start=False,
                        stop=True,
                    )
                    nc.scalar.activation(
                        h18v[:, hc, hlo : hlo + W], ps[:, :], AF.Relu,
                        scale=1.0 / 16.0,
                    )
                # ----- layer 3 + sampling (this portion), per row tile -----
                for bt in range(b0, b1):
                    logits_bt = mpool.tile(
                        [128, NACT], f32, tag="logits", bufs=3, name=f"logits{bt}"
                    )
                    for cc in range(3):
                        ps = pmm_pool.tile([128, CW], f32, tag="pmm")
                        for dd in range(4):
                            nc.tensor.matmul(
                                ps[:, :],
                                h18v[:, 2 * dd : 2 * dd + 2, bt * 128 : (bt + 1) * 128],
                                w2f8v[:, 2 * dd : 2 * dd + 2, cc * CW : (cc + 1) * CW],
                                start=(dd == 0),
                                stop=False,
                                perf_mode=DR,
                            )
                        nc.tensor.matmul(
                            ps[:, :],
                            ones_row[:, bt * 128 : (bt + 1) * 128],
                            w2tail[:, cc * CW : (cc + 1) * CW],
                            start=False,
                            stop=True,
                        )
                        nc.scalar.mul(
                            logits_bt[:, cc * CW : (cc + 1) * CW], ps[:, :],
                            1.0 / 16.0,
                        )
                    lsl = logits_bt[:, :]
                    # head sums: Z0 = sum exp, S0 = sum exp*logit
                    etile = epool.tile([128, NACT], f32, tag="etile")
                    for h in range(2):
                        f = bt * 2 + h
                        nc.scalar.activation(
                            etile[:, h * NH : (h + 1) * NH],
                            lsl[:, h * NH : (h + 1) * NH],
                            AF.Exp,
                            accum_out=z0[:, f : f + 1],
                        )
                        nc.vector.tensor_tensor(
                            etile[:, h * NH : (h + 1) * NH],
                            etile[:, h * NH : (h + 1) * NH],
                            lsl[:, h * NH : (h + 1) * NH],
                            OP.mult,
                        )
                        nc.vector.tensor_reduce(
                            s0[:, f : f + 1],
                            etile[:, h * NH : (h + 1) * NH],
                            AX.X,
                            OP.add,
                        )
                    # gather this row tile's chosen logits
                    graw = mpool.tile([128, CS * 16], f32, tag="grawc", bufs=2)
                    ltouch = lsl.rearrange("p (x c) -> p x c", c=CW)[:, :, 0:1]
                    nc.gpsimd.tensor_copy(
                        graw[:, 0:3].rearrange("p (x c) -> p x c", c=1), ltouch
                    )
                    nc.gpsimd.tensor_copy(
                        iscr[:, :], idxs_sb[:, bt * CS : bt * CS + 1]
                    )
                    nc.gpsimd.ap_gather(
                        out_ap=graw[:, :],
                        in_ap=lsl,
                        idxs_ap=idxs_sb[:, bt * CS : (bt + 1) * CS],
                        channels=128,
                        num_elems=NACT,
                        d=1,
                        num_idxs=CS * 16,
                    )
                    graw3 = graw[:, :].rearrange("p (s q) -> p s q", q=16)
                    diag3 = diag_sb[:, :].rearrange("p q -> p () q").broadcast_to(
                        (128, CS, 16)
                    )
                    nc.gpsimd.tensor_tensor(graw3, graw3, diag3, OP.mult)
                    nc.vector.tensor_reduce(
                        gsel[:, bt * CS : (bt + 1) * CS], graw3, AX.X, OP.add
                    )
                    if b1 - b0 == 4:
                        # wide portion: pair row-tiles to halve DVE op count
                        if bt % 2 == 1:
                            recurrence_chunk(bt - 1, span=2)
                    else:
                        recurrence_chunk(bt)


    nc.compile()
    return nc


def _get_nc():
    if "nc" not in _CACHE:
        _CACHE["nc"] = _build_nc()
    return _CACHE["nc"]


def _prep_shared(W0, b0, W1, b1, W2, b2):
    import ml_dtypes

    bf = ml_dtypes.bfloat16
    f8 = ml_dtypes.float8_e4m3
    FS = 16.0  # fp8 weight pre-scale; undone by the relu-copy's 1/16
    w0e8 = (
        (W0[:1024] * FS).reshape(8, 128, H).transpose(1, 0, 2).reshape(128, 8 * H)
    ).astype(f8)
    w0et = np.vstack([W0[1024:1025] * FS, b0[None, :] * FS]).astype(bf)  # (2, H)
    w1e8 = (
        (W1 * FS).reshape(8, 128, H).transpose(1, 0, 2).reshape(128, 8 * H)
    ).astype(f8)
    w1et = (b1[None, :] * FS).astype(bf)                     # (1, H)
    w2e8 = (
        (W2 * FS).reshape(8, 128, NACT).transpose(1, 0, 2).reshape(128, 8 * NACT)
    ).astype(f8)
    w2et = (b2[None, :] * FS).astype(bf)                     # (1, NACT)
    t = np.arange(K, dtype=np.int32)
    reset = np.broadcast_to(
        (t != 0).astype(np.float32), (128, NSEG, K)
    ).reshape(128, SFREE).copy()
    q = np.arange(16)
    diag = (q[None, :] == (np.arange(128) % 16)[:, None]).astype(np.float32)
    return (w0e8, w0et, w1e8, w1et, w2e8, w2et), reset, diag


def _prep_core(c, state, seq_idx_R, seq_len_R, seq_idx_S, seq_len_S):
    import ml_dtypes

    bf = ml_dtypes.bfloat16
    rows = slice(c * BC, (c + 1) * BC)
    st = state[rows].astype(np.float32)
    statee = np.concatenate(
        [st, np.ones((BC, 1), np.float32)], axis=1
    ).astype(bf)                                             # (1024, 1026)

    idxR = np.maximum(seq_idx_R[rows].astype(np.int32), 0)
    idxS = np.maximum(seq_idx_S[rows].astype(np.int32), 0) + NH
    idx_all = np.stack([idxR, idxS], axis=1)                 # (1024, 2, 32)
    enc = idx_all.reshape(R, 128, 2, K)
    idxs = enc.transpose(1, 0, 2, 3).reshape(128, SFREE).astype(np.int16)

    lens = np.stack([seq_len_R[rows], seq_len_S[rows]], axis=1)  # (1024, 2)
    t = np.arange(K, dtype=np.int32)
    act = (t[None, None, :] < lens[:, :, None]).astype(np.float32)
    mmask = (
        act.reshape(R, 128, 2, K).transpose(1, 0, 2, 3).reshape(128, SFREE)
    ).astype(np.float32)
    return statee, idxs, mmask


def kernel(
    state,
    W0,
    b0,
    W1,
    b1,
    W2,
    b2,
    seq_idx_R,
    seq_len_R,
    seq_idx_S,
    seq_len_S,
    _return_extras=False,
):
    from concourse.bass_utils import run_bass_kernel_spmd

    nc = _get_nc()
    (w0e8, w0et, w1e8, w1et, w2e8, w2et), reset, diag = _prep_shared(
        np.asarray(W0), np.asarray(b0), np.asarray(W1),
        np.asarray(b1), np.asarray(W2), np.asarray(b2),
    )
    in_maps = []
    for c in range(NCORES):
        statee, idxs, mmask = _prep_core(
            c,
            np.asarray(state),
            np.asarray(seq_idx_R),
            np.asarray(seq_len_R),
            np.asarray(seq_idx_S),
            np.asarray(seq_len_S),
        )
        in_maps.append(
            {
                "statee": statee,
                "w0e8": w0e8,
                "w0et": w0et,
                "w1e8": w1e8,
                "w1et": w1et,
                "w2e8": w2e8,
                "w2et": w2et,
                "idxs": idxs,
                "mmask": mmask,
                "reset": reset,
                "diag": diag,
            }
        )
    res = run_bass_kernel_spmd(nc, in_maps, core_ids=list(range(NCORES)))
    out = np.concatenate(
        [res.results[c]["out"].astype(np.float32) for c in range(NCORES)], axis=1
    )
    if _return_extras:
        return out, res
    return out
